# Initial kernel scaffold
#
"""Single-head attention (B=4, S=2048, D=1024) on 8 Trainium2 NeuronCores.

Sharding: batch x KEY-half. Core c handles batch b=c//2 and key rows
[1024*h : 1024*(h+1)] with h=c%2. Each core receives x[b] rolled so its own
key rows come first; it computes Q for ALL 2048 (rolled) queries, K/V for its
1024 keys, and outputs the UNNORMALIZED partial attention O~ = exp(S)V plus
partial row-sums. The host un-rolls the query order and combines the pair:
O = (O~_0 + O~_1) / (rs_0 + rs_1).  (No softmax max-subtraction is needed:
scaled scores are ~N(0,1), so exp never overflows, and partials add.)

All matmul operands are bf16 (PSUM accumulation stays fp32): on TRN2 the PE
streams bf16 moving operands at 1 col/cycle vs ~1.75 for fp32r, and bf16
weights get fast-weight-load. bf16 also halves SBUF footprints, so Q^T stays
fully resident (no DRAM spill) and x^T loads as 4MB instead of 8MB.
Measured end-to-end rel err ~5e-3 (gate 2e-2).

Per-core pipeline (activations kept [feature, token] transposed so the PE
contracts over partitions):
  K:  K^T[ec] = Wk[:,ec-blk].T @ x^T[:, 0:1024]  (own keys first after roll)
  V:  V[kc]   = x^T[:, kc-blk].T @ Wv            (natural [k, e]; bias via a
      precomputed broadcast tile + DVE fused add on PSUM evacuation)
  Q:  Q^T[ec] = Wq[:,ec-blk].T @ x^T             (all 2048 queries, resident)
  C:  per 512-query block: S^T[k,q] = K^T.T @ Q^T; exp on ACT writes P^T
      straight to SBUF as bf16; partial row-sums via ones-vector matmuls;
      O~ = P^T.T @ V; DMA out raw.
Host pre-transposes/casts x and pre-shapes the weights so every DMA is
contiguous 2KB+ per partition line.
"""

import sys
from contextlib import ExitStack

import ml_dtypes
import numpy as np

if "/opt/trn_rl_repo" not in sys.path:
    sys.path.insert(0, "/opt/trn_rl_repo")

import concourse.bass as bass
import concourse.bacc as bacc
import concourse.tile as tile
from concourse import mybir
from concourse.bass_utils import run_bass_kernel_spmd

P = 128
S = 2048        # full sequence (queries per core)
SK = 1024       # keys per core (own half)
D = 1024        # model dim
F32 = mybir.dt.float32
BF16 = mybir.dt.bfloat16
NPBF16 = ml_dtypes.bfloat16

DC = D // P     # 8 d-chunks (contraction over model dim)
EC = D // P     # 8 e-chunks (output features)
KC = SK // P    # 8 key chunks (own half)
NT = 512        # moving-operand tile (one PSUM bank of fp32)
QT = 512        # query tile in phase C

SCALE = 1.0 / float(np.sqrt(np.float32(D)))
Identity = mybir.ActivationFunctionType.Identity
Exp = mybir.ActivationFunctionType.Exp


def build_program() -> bass.Bass:
    nc = bacc.Bacc(
        "TRN2", target_bir_lowering=False, debug=False, num_devices=8)

    xT_d = nc.dram_tensor("xT", [S // NT, P, DC, NT], BF16,
                      kind="ExternalInput").ap()
    wq_d = nc.dram_tensor("Wqr", [EC, P, DC, P], BF16, kind="ExternalInput").ap()
    wk_d = nc.dram_tensor("Wkr", [EC, P, DC, P], BF16, kind="ExternalInput").ap()
    wv_d = nc.dram_tensor("Wvr", [D, D], BF16, kind="ExternalInput").ap()
    bq_d = nc.dram_tensor("bqr", [P, EC], F32, kind="ExternalInput").ap()
    bk_d = nc.dram_tensor("bkr", [P, EC], F32, kind="ExternalInput").ap()
    bv_d = nc.dram_tensor("bvr", [1, D], BF16, kind="ExternalInput").ap()
    o_d = nc.dram_tensor("o_raw", [S, D], BF16, kind="ExternalOutput").ap()
    rs_d = nc.dram_tensor("rs_raw", [S // QT, P, QT // P], F32,
                      kind="ExternalOutput").ap()

    with tile.TileContext(nc) as tc, ExitStack() as ctx:
        const_p = ctx.enter_context(tc.tile_pool(name="const", bufs=1))
        xt_p = ctx.enter_context(tc.tile_pool(name="xt", bufs=S // NT))
        kt_p = ctx.enter_context(tc.tile_pool(name="kt", bufs=EC))
        qt_p = ctx.enter_context(tc.tile_pool(name="qt", bufs=EC))
        v_p = ctx.enter_context(tc.tile_pool(name="v", bufs=KC))
        wk_p = ctx.enter_context(tc.tile_pool(name="wk", bufs=EC))
        wq_p = ctx.enter_context(tc.tile_pool(name="wq", bufs=EC))
        wv_p = ctx.enter_context(tc.tile_pool(name="wv", bufs=DC))
        pt_p = ctx.enter_context(tc.tile_pool(name="ptp", bufs=12))
        osb_p = ctx.enter_context(tc.tile_pool(name="osb", bufs=3))
        st_p = ctx.enter_context(tc.tile_pool(name="stat", bufs=2))
        psA = ctx.enter_context(tc.tile_pool(name="psA", bufs=4, space="PSUM"))
        psB = ctx.enter_context(tc.tile_pool(name="psB", bufs=3, space="PSUM"))
        psR = ctx.enter_context(tc.tile_pool(name="psR", bufs=1, space="PSUM"))

        # ---- DMA issue order --------------------------------------------
        # Two queues race the K phase's needs. gpsimd (SWDGE) starts issuing
        # ~1.3us before sync: it carries x block 0 (the first chain's gating
        # input), the biases, and Wv. sync (HWDGE) carries Wk (first two
        # blocks ahead of x block 1 so chain ec=0/kb=1 isn't starved), the
        # remaining x blocks, and Wq. x is host-pre-arranged [tb, p, dc, t]
        # so each 1MB token block is one fully-contiguous DMA (8KB lines).
        xt = [xt_p.tile([P, DC, NT], BF16, name=f"xt{tb}", tag="xt")
              for tb in range(S // NT)]
        ones_col = const_p.tile([P, 1], BF16)   # lhsT for row-sums
        nc.vector.memset(ones_col[:], 1.0)
        ones_row = const_p.tile([1, P], BF16)   # rank-1 bv broadcast
        nc.vector.memset(ones_row[:], 1.0)
        bvb = const_p.tile([P, D], F32)         # bv broadcast across parts

        nc.gpsimd.dma_start(xt[0][P // 2:P], xT_d[0][P // 2:P])
        bvr = const_p.tile([1, D], BF16)
        nc.gpsimd.dma_start(bvr[:], bv_d[:])
        bqt = const_p.tile([P, EC], F32)
        nc.gpsimd.dma_start(bqt[:], bq_d[:])
        bkt = const_p.tile([P, EC], F32)
        nc.gpsimd.dma_start(bkt[:], bk_d[:])
        wv = [wv_p.tile([P, D], BF16, name=f"wv{dc}", tag="wv")
              for dc in range(DC)]
        for dc in range(DC):
            nc.gpsimd.dma_start(wv[dc][:], wv_d[dc * P:(dc + 1) * P, :])

        wk = [wk_p.tile([P, DC, P], BF16, name=f"wk{ec}", tag="wk")
              for ec in range(EC)]
        nc.sync.dma_start(xt[0][0:P // 2], xT_d[0][0:P // 2])
        for ec in range(EC):
            nc.sync.dma_start(wk[ec][:], wk_d[ec])
        for tb in range(1, S // NT):
            nc.sync.dma_start(xt[tb][:], xT_d[tb])
        wq = [wq_p.tile([P, DC, P], BF16, name=f"wq{ec}", tag="wq")
              for ec in range(EC)]
        for ec in range(EC):
            nc.sync.dma_start(wq[ec][:], wq_d[ec])

        # PE warm-up during the initial DMA window: dummy matmuls on a
        # memset scratch tile get the HAM clock gate to K=8/8 before the
        # first real chain. Every PSUM group gets a DVE reader (narrow
        # copy) — matmul groups with no consumer have wedged the device.
        scr = const_p.tile([P, NT], BF16)
        nc.vector.memset(scr[:], 0.0)
        scr_out = const_p.tile([P, NT], F32)
        for i in range(8):
            pool = psA if i % 2 == 0 else psB
            ps = pool.tile([P, NT], F32)
            nc.tensor.matmul(
                ps[:], scr[:, 0:P], scr[:], start=True, stop=True)
            # reader on the idle ACT engine: the DVE queue's startup traffic
            # has been seen delaying these reads 3us, stalling the warm-up
            nc.scalar.activation(
                scr_out[:, i * 64:(i + 1) * 64], ps[:, 0:64], Identity)

        # bv broadcast tile: rank-1 ones_row^T @ bv_row, evac to f32 SBUF.
        # Emitted first on the PE: runs during the initial DMA window.
        for et in range(D // NT):
            ps = psB.tile([P, NT], F32)
            nc.tensor.matmul(
                ps[:], ones_row[0:1, :], bvr[0:1, et * NT:(et + 1) * NT],
                start=True, stop=True)
            nc.vector.tensor_copy(bvb[:, et * NT:(et + 1) * NT], ps[:])

        # ---- Phase K: K^T (own keys) resident ---------------------------
        kt = [kt_p.tile([P, SK], BF16, name=f"kt{ec}", tag="kt")
              for ec in range(EC)]
        for kb in range(SK // NT):
            for ec in range(EC):
                ps = psA.tile([P, NT], F32)
                for dc in range(DC):
                    nc.tensor.matmul(
                        ps[:],
                        wk[ec][:, dc, :],
                        xt[kb][:, dc, :],
                        start=(dc == 0), stop=(dc == DC - 1),
                    )
                nc.scalar.activation(
                    kt[ec][:, kb * NT:(kb + 1) * NT], ps[:], Identity,
                    bias=bkt[:, ec:ec + 1],
                )

        # ---- Phase V: V natural [k, e] (own keys) resident --------------
        v = [v_p.tile([P, D], BF16, name=f"v{kc}", tag="v") for kc in range(KC)]
        for kc in range(KC):
            for et in range(D // NT):
                ps = psB.tile([P, NT], F32)
                tb, j = divmod(kc, NT // P)
                for dc in range(DC):
                    nc.tensor.matmul(
                        ps[:],
                        xt[tb][:, dc, j * P:(j + 1) * P],
                        wv[dc][:, et * NT:(et + 1) * NT],
                        start=(dc == 0), stop=(dc == DC - 1),
                    )
                # fused bias add on evacuation: v = ps + bvb
                nc.vector.scalar_tensor_tensor(
                    v[kc][:, et * NT:(et + 1) * NT], ps[:], 1.0,
                    bvb[:, et * NT:(et + 1) * NT],
                    mybir.AluOpType.mult, mybir.AluOpType.add,
                )

        # ---- Phase Q: Q^T (all queries) resident ------------------------
        qt = [qt_p.tile([P, S], BF16, name=f"qt{ec}", tag="qt")
              for ec in range(EC)]
        for ec in range(EC):
            for qb in range(S // NT):
                ps = psA.tile([P, NT], F32)
                for dc in range(DC):
                    nc.tensor.matmul(
                        ps[:],
                        wq[ec][:, dc, :],
                        xt[qb][:, dc, :],
                        start=(dc == 0), stop=(dc == DC - 1),
                    )
                nc.scalar.activation(
                    qt[ec][:, qb * NT:(qb + 1) * NT], ps[:], Identity,
                    bias=bqt[:, ec:ec + 1],
                )

        # ---- Phase C: attention, transposed scores ----------------------
        for qq in range(S // QT):
            # S^T[k, q] per key chunk; exp writes P^T straight to SBUF bf16
            ptt = [pt_p.tile([P, QT], BF16, tag="ptp", name=f"ptt{kc}")
                   for kc in range(KC)]
            for kc in range(KC):
                ps = psA.tile([P, QT], F32)
                for ec in range(EC):
                    nc.tensor.matmul(
                        ps[:],
                        kt[ec][:, kc * P:(kc + 1) * P],
                        qt[ec][:, qq * QT:(qq + 1) * QT],
                        start=(ec == 0), stop=(ec == EC - 1),
                    )
                nc.scalar.activation(ptt[kc][:], ps[:], Exp, scale=SCALE)

            # O~ = P^T.T @ V per 128-query chunk, with the partial row-sum
            # fused in: after each et=0 matmul, an N=1 matmul with the SAME
            # stationary P^T block and a moving ones-column accumulates
            # rs[q] — its LDWEIGHTS hides under the 512-wide stream, so this
            # replaces the separate 8-matmul row-sum chain almost for free.
            rs_cols = st_p.tile([P, QT // P], F32, name="rsc", tag="rs")
            last_qq = qq == S // QT - 1
            for qc in range(QT // P):
                last_blk = last_qq and qc == QT // P - 1
                o_sb = osb_p.tile([P, D], BF16, name="osb", tag="osb")
                psq = psR.tile([P, 1], F32, name="psq")
                row0 = qq * QT + qc * P
                for et in range(D // NT):
                    ps = psB.tile([P, NT], F32)
                    for kc in range(KC):
                        nc.tensor.matmul(
                            ps[:],
                            ptt[kc][:, qc * P:(qc + 1) * P],
                            v[kc][:, et * NT:(et + 1) * NT],
                            start=(kc == 0), stop=(kc == KC - 1),
                        )
                        if et == 0:
                            nc.tensor.matmul(
                                psq[:],
                                ptt[kc][:, qc * P:(qc + 1) * P],
                                ones_col[:],
                                start=(kc == 0), stop=(kc == KC - 1),
                            )
                    nc.vector.tensor_copy(
                        o_sb[:, et * NT:(et + 1) * NT], ps[:])
                    if last_blk:
                        # drain the kernel tail: ship each half as soon as
                        # it's evacuated, with the tiny rs DMA in between
                        if et == 0:
                            nc.vector.tensor_copy(
                                rs_cols[:, qc:qc + 1], psq[:])
                            nc.sync.dma_start(
                                o_d[row0:row0 + P, 0:NT], o_sb[:, 0:NT])
                            nc.sync.dma_start(rs_d[qq], rs_cols[:])
                        else:
                            nc.sync.dma_start(
                                o_d[row0:row0 + P, NT:D], o_sb[:, NT:D])
                if not last_blk:
                    nc.vector.tensor_copy(rs_cols[:, qc:qc + 1], psq[:])
                    nc.sync.dma_start(o_d[row0:row0 + P, :], o_sb[:])
            if not last_qq:
                nc.sync.dma_start(rs_d[qq], rs_cols[:])

    nc.compile()
    return nc


_CACHE: dict = {}


def _get_program() -> bass.Bass:
    if "nc" not in _CACHE:
        _CACHE["nc"] = build_program()
    return _CACHE["nc"]


def kernel(x, Wq, bq, Wk, bk, Wv, bv, _trace=False, _trace_kwargs=None):
    nc = _get_program()
    x = np.asarray(x, dtype=np.float32)

    def wrearr(w):
        # [d, e] -> [ec, p(d%128), dc, e%128] so each ec-block DMA is
        # contiguous with 2KB per partition line
        w = np.asarray(w, np.float32).astype(NPBF16)
        return np.ascontiguousarray(
            w.reshape(DC, P, EC, P).transpose(2, 1, 0, 3))

    def brearr(b):
        b = np.asarray(b, np.float32)
        return np.ascontiguousarray(b.reshape(EC, P).T)

    shared = {
        "Wqr": wrearr(Wq),
        "Wkr": wrearr(Wk),
        "Wvr": np.ascontiguousarray(np.asarray(Wv, np.float32).astype(NPBF16)),
        "bqr": brearr(bq),
        "bkr": brearr(bk),
        "bvr": np.ascontiguousarray(
            np.asarray(bv, np.float32).astype(NPBF16).reshape(1, D)),
    }
    in_maps = []
    for c in range(8):
        b, h = divmod(c, 2)
        xb = x[b]
        if h:
            xb = np.roll(xb, -SK, axis=0)  # own key half first
        xTb = xb.T.astype(NPBF16)  # [D, S]
        # [d, s] -> [tb, p, dc, t]: each 512-token block fully contiguous
        xTr = np.ascontiguousarray(
            xTb.reshape(DC, P, S // NT, NT).transpose(2, 1, 0, 3))
        in_maps.append({"xT": xTr, **shared})

    res = run_bass_kernel_spmd(
        nc, in_maps, list(range(8)),
        trace=_trace, **(_trace_kwargs or {}),
    )
    out = np.empty((4, S, D), dtype=np.float32)
    for b in range(4):
        def unrs(r):
            # device writes rs as [qq, p, qc] = rowsum(qq*512 + qc*128 + p)
            return r.reshape(S // QT, P, QT // P).transpose(
                0, 2, 1).reshape(S).astype(np.float64)

        o0 = res.results[2 * b]["o_raw"].astype(np.float64)
        r0 = unrs(res.results[2 * b]["rs_raw"])
        o1 = res.results[2 * b + 1]["o_raw"].astype(np.float64)
        r1 = unrs(res.results[2 * b + 1]["rs_raw"])
        # core h=1 computed queries in rolled order; un-roll before combining
        o1 = np.roll(o1, SK, axis=0)
        r1 = np.roll(r1, SK)
        out[b] = ((o0 + o1) / (r0 + r1)[:, None]).astype(np.float32)
    if _trace:
        return out, res
    return out



# revision 1
# speedup vs baseline: 1.2419x; 1.2419x over previous
"""Single-head attention (B=4, S=2048, D=1024) on 8 Trainium2 NeuronCores.

Sharding: batch x KEY-half. Core c handles batch b=c//2 and key rows
[1024*h : 1024*(h+1)] with h=c%2. Each core receives x[b] rolled so its own
key rows come first; it computes Q for ALL 2048 (rolled) queries, K/V for its
1024 keys, and outputs the UNNORMALIZED partial attention O~ = exp(S)V plus
partial row-sums. The host un-rolls the query order and combines the pair:
O = (O~_0 + O~_1) / (rs_0 + rs_1).  (No softmax max-subtraction is needed:
scaled scores are ~N(0,1), so exp never overflows, and partials add.)

All matmul operands are bf16 (PSUM accumulation stays fp32): on TRN2 the PE
streams bf16 moving operands at 1 col/cycle vs ~1.75 for fp32r, and bf16
weights get fast-weight-load. bf16 also halves SBUF footprints, so Q^T stays
fully resident (no DRAM spill) and x^T loads as 4MB instead of 8MB.
Measured end-to-end rel err ~5e-3 (gate 2e-2).

Per-core pipeline (activations kept [feature, token] transposed so the PE
contracts over partitions):
  K:  K^T[ec] = Wk[:,ec-blk].T @ x^T[:, 0:1024]  (own keys first after roll)
  V:  V[kc]   = x^T[:, kc-blk].T @ Wv            (natural [k, e]; bias via a
      precomputed broadcast tile + DVE fused add on PSUM evacuation)
  Q:  Q^T[ec] = Wq[:,ec-blk].T @ x^T             (all 2048 queries, resident)
  C:  per 512-query block: S^T[k,q] = K^T.T @ Q^T; exp on ACT writes P^T
      straight to SBUF as bf16; partial row-sums via ones-vector matmuls;
      O~ = P^T.T @ V; DMA out raw.
Host pre-transposes/casts x and pre-shapes the weights so every DMA is
contiguous 2KB+ per partition line.
"""

import sys
from contextlib import ExitStack

import ml_dtypes
import numpy as np

if "/opt/trn_rl_repo" not in sys.path:
    sys.path.insert(0, "/opt/trn_rl_repo")

import concourse.bass as bass
import concourse.bacc as bacc
import concourse.tile as tile
from concourse import mybir
from concourse.bass_utils import run_bass_kernel_spmd

P = 128
S = 2048        # full sequence (queries per core)
SK = 1024       # keys per core (own half)
D = 1024        # model dim
F32 = mybir.dt.float32
BF16 = mybir.dt.bfloat16
NPBF16 = ml_dtypes.bfloat16

DC = D // P     # 8 d-chunks (contraction over model dim)
EC = D // P     # 8 e-chunks (output features)
KC = SK // P    # 8 key chunks (own half)
NT = 512        # moving-operand tile (one PSUM bank of fp32)
QT = 512        # query tile in phase C

SCALE = 1.0 / float(np.sqrt(np.float32(D)))
Identity = mybir.ActivationFunctionType.Identity
Exp = mybir.ActivationFunctionType.Exp


def build_program() -> bass.Bass:
    nc = bacc.Bacc(
        "TRN2", target_bir_lowering=False, debug=False, num_devices=8)

    xT_d = nc.dram_tensor("xT", [S // NT, P, DC, NT], BF16,
                      kind="ExternalInput").ap()
    wq_d = nc.dram_tensor("Wqr", [EC, P, DC, P], BF16, kind="ExternalInput").ap()
    wk_d = nc.dram_tensor("Wkr", [EC, P, DC, P], BF16, kind="ExternalInput").ap()
    wv_d = nc.dram_tensor("Wvr", [D, D], BF16, kind="ExternalInput").ap()
    bq_d = nc.dram_tensor("bqr", [P, EC], F32, kind="ExternalInput").ap()
    bk_d = nc.dram_tensor("bkr", [P, EC], F32, kind="ExternalInput").ap()
    bv_d = nc.dram_tensor("bvr", [1, D], BF16, kind="ExternalInput").ap()
    o_d = nc.dram_tensor("o_raw", [S, D], BF16, kind="ExternalOutput").ap()
    rs_d = nc.dram_tensor("rs_raw", [S // QT, P, QT // P], F32,
                      kind="ExternalOutput").ap()

    with tile.TileContext(nc) as tc, ExitStack() as ctx:
        const_p = ctx.enter_context(tc.tile_pool(name="const", bufs=1))
        xt_p = ctx.enter_context(tc.tile_pool(name="xt", bufs=S // NT))
        kt_p = ctx.enter_context(tc.tile_pool(name="kt", bufs=EC))
        qt_p = ctx.enter_context(tc.tile_pool(name="qt", bufs=EC))
        v_p = ctx.enter_context(tc.tile_pool(name="v", bufs=KC))
        wk_p = ctx.enter_context(tc.tile_pool(name="wk", bufs=EC))
        wq_p = ctx.enter_context(tc.tile_pool(name="wq", bufs=EC))
        wv_p = ctx.enter_context(tc.tile_pool(name="wv", bufs=DC))
        pt_p = ctx.enter_context(tc.tile_pool(name="ptp", bufs=12))
        osb_p = ctx.enter_context(tc.tile_pool(name="osb", bufs=3))
        st_p = ctx.enter_context(tc.tile_pool(name="stat", bufs=2))
        psA = ctx.enter_context(tc.tile_pool(name="psA", bufs=4, space="PSUM"))
        psB = ctx.enter_context(tc.tile_pool(name="psB", bufs=3, space="PSUM"))
        psR = ctx.enter_context(tc.tile_pool(name="psR", bufs=1, space="PSUM"))

        # ---- DMA issue order --------------------------------------------
        # Two queues race the K phase's needs. gpsimd (SWDGE) starts issuing
        # ~1.3us before sync: it carries x block 0 (the first chain's gating
        # input), the biases, and Wv. sync (HWDGE) carries Wk (first two
        # blocks ahead of x block 1 so chain ec=0/kb=1 isn't starved), the
        # remaining x blocks, and Wq. x is host-pre-arranged [tb, p, dc, t]
        # so each 1MB token block is one fully-contiguous DMA (8KB lines).
        xt = [xt_p.tile([P, DC, NT], BF16, name=f"xt{tb}", tag="xt")
              for tb in range(S // NT)]
        ones_col = const_p.tile([P, 1], BF16)   # lhsT for row-sums
        nc.vector.memset(ones_col[:], 1.0)
        ones_row = const_p.tile([1, P], BF16)   # rank-1 bv broadcast
        nc.vector.memset(ones_row[:], 1.0)
        bvb = const_p.tile([P, D], F32)         # bv broadcast across parts

        nc.gpsimd.dma_start(xt[0][P // 2:P], xT_d[0][P // 2:P])
        bvr = const_p.tile([1, D], BF16)
        nc.gpsimd.dma_start(bvr[:], bv_d[:])
        bqt = const_p.tile([P, EC], F32)
        nc.gpsimd.dma_start(bqt[:], bq_d[:])
        bkt = const_p.tile([P, EC], F32)
        nc.gpsimd.dma_start(bkt[:], bk_d[:])
        wv = [wv_p.tile([P, D], BF16, name=f"wv{dc}", tag="wv")
              for dc in range(DC)]
        for dc in range(DC):
            nc.gpsimd.dma_start(wv[dc][:], wv_d[dc * P:(dc + 1) * P, :])

        wk = [wk_p.tile([P, DC, P], BF16, name=f"wk{ec}", tag="wk")
              for ec in range(EC)]
        nc.sync.dma_start(xt[0][0:P // 2], xT_d[0][0:P // 2])
        for ec in range(EC):
            nc.sync.dma_start(wk[ec][:], wk_d[ec])
        for tb in range(1, S // NT):
            nc.sync.dma_start(xt[tb][:], xT_d[tb])
        wq = [wq_p.tile([P, DC, P], BF16, name=f"wq{ec}", tag="wq")
              for ec in range(EC)]
        for ec in range(EC):
            nc.sync.dma_start(wq[ec][:], wq_d[ec])

        # PE warm-up during the initial DMA window: dummy matmuls on a
        # memset scratch tile get the HAM clock gate to K=8/8 before the
        # first real chain. Every PSUM group gets a DVE reader (narrow
        # copy) — matmul groups with no consumer have wedged the device.
        scr = const_p.tile([P, NT], BF16)
        nc.vector.memset(scr[:], 0.0)
        scr_out = const_p.tile([P, NT], F32)
        for i in range(8):
            pool = psA if i % 2 == 0 else psB
            ps = pool.tile([P, NT], F32)
            nc.tensor.matmul(
                ps[:], scr[:, 0:P], scr[:], start=True, stop=True)
            # reader on the idle ACT engine: the DVE queue's startup traffic
            # has been seen delaying these reads 3us, stalling the warm-up
            nc.scalar.activation(
                scr_out[:, i * 64:(i + 1) * 64], ps[:, 0:64], Identity)

        # bv broadcast tile: rank-1 ones_row^T @ bv_row, evac to f32 SBUF.
        # Emitted first on the PE: runs during the initial DMA window.
        for et in range(D // NT):
            ps = psB.tile([P, NT], F32)
            nc.tensor.matmul(
                ps[:], ones_row[0:1, :], bvr[0:1, et * NT:(et + 1) * NT],
                start=True, stop=True)
            nc.vector.tensor_copy(bvb[:, et * NT:(et + 1) * NT], ps[:])

        # ---- Phase K: K^T (own keys) resident ---------------------------
        kt = [kt_p.tile([P, SK], BF16, name=f"kt{ec}", tag="kt")
              for ec in range(EC)]
        for kb in range(SK // NT):
            for ec in range(EC):
                ps = psA.tile([P, NT], F32)
                for dc in range(DC):
                    nc.tensor.matmul(
                        ps[:],
                        wk[ec][:, dc, :],
                        xt[kb][:, dc, :],
                        start=(dc == 0), stop=(dc == DC - 1),
                    )
                nc.scalar.activation(
                    kt[ec][:, kb * NT:(kb + 1) * NT], ps[:], Identity,
                    bias=bkt[:, ec:ec + 1],
                )

        # ---- Phase V: V natural [k, e] (own keys) resident --------------
        v = [v_p.tile([P, D], BF16, name=f"v{kc}", tag="v") for kc in range(KC)]
        for kc in range(KC):
            for et in range(D // NT):
                ps = psB.tile([P, NT], F32)
                tb, j = divmod(kc, NT // P)
                for dc in range(DC):
                    nc.tensor.matmul(
                        ps[:],
                        xt[tb][:, dc, j * P:(j + 1) * P],
                        wv[dc][:, et * NT:(et + 1) * NT],
                        start=(dc == 0), stop=(dc == DC - 1),
                    )
                # fused bias add on evacuation: v = ps + bvb
                nc.vector.scalar_tensor_tensor(
                    v[kc][:, et * NT:(et + 1) * NT], ps[:], 1.0,
                    bvb[:, et * NT:(et + 1) * NT],
                    mybir.AluOpType.mult, mybir.AluOpType.add,
                )

        # ---- Phase Q: Q^T (all queries) resident ------------------------
        qt = [qt_p.tile([P, S], BF16, name=f"qt{ec}", tag="qt")
              for ec in range(EC)]
        for ec in range(EC):
            for qb in range(S // NT):
                ps = psA.tile([P, NT], F32)
                for dc in range(DC):
                    nc.tensor.matmul(
                        ps[:],
                        wq[ec][:, dc, :],
                        xt[qb][:, dc, :],
                        start=(dc == 0), stop=(dc == DC - 1),
                    )
                nc.scalar.activation(
                    qt[ec][:, qb * NT:(qb + 1) * NT], ps[:], Identity,
                    bias=bqt[:, ec:ec + 1],
                )

        # ---- Phase C: attention, transposed scores ----------------------
        for qq in range(S // QT):
            # S^T[k, q] per key chunk; exp writes P^T straight to SBUF bf16
            ptt = [pt_p.tile([P, QT], BF16, tag="ptp", name=f"ptt{kc}")
                   for kc in range(KC)]
            for kc in range(KC):
                ps = psA.tile([P, QT], F32)
                for ec in range(EC):
                    nc.tensor.matmul(
                        ps[:],
                        kt[ec][:, kc * P:(kc + 1) * P],
                        qt[ec][:, qq * QT:(qq + 1) * QT],
                        start=(ec == 0), stop=(ec == EC - 1),
                    )
                nc.scalar.activation(ptt[kc][:], ps[:], Exp, scale=SCALE)

            # O~ = P^T.T @ V per 128-query chunk, with the partial row-sum
            # fused in: after each et=0 matmul, an N=1 matmul with the SAME
            # stationary P^T block and a moving ones-column accumulates
            # rs[q] — its LDWEIGHTS hides under the 512-wide stream, so this
            # replaces the separate 8-matmul row-sum chain almost for free.
            rs_cols = st_p.tile([P, QT // P], F32, name="rsc", tag="rs")
            last_qq = qq == S // QT - 1
            for qc in range(QT // P):
                last_blk = last_qq and qc == QT // P - 1
                o_sb = osb_p.tile([P, D], BF16, name="osb", tag="osb")
                psq = psR.tile([P, 1], F32, name="psq")
                row0 = qq * QT + qc * P
                for et in range(D // NT):
                    ps = psB.tile([P, NT], F32)
                    for kc in range(KC):
                        nc.tensor.matmul(
                            ps[:],
                            ptt[kc][:, qc * P:(qc + 1) * P],
                            v[kc][:, et * NT:(et + 1) * NT],
                            start=(kc == 0), stop=(kc == KC - 1),
                        )
                        if et == 0:
                            nc.tensor.matmul(
                                psq[:],
                                ptt[kc][:, qc * P:(qc + 1) * P],
                                ones_col[:],
                                start=(kc == 0), stop=(kc == KC - 1),
                            )
                    nc.vector.tensor_copy(
                        o_sb[:, et * NT:(et + 1) * NT], ps[:])
                    if last_blk:
                        # drain the kernel tail: ship each half as soon as
                        # it's evacuated, with the tiny rs DMA in between
                        if et == 0:
                            nc.vector.tensor_copy(
                                rs_cols[:, qc:qc + 1], psq[:])
                            nc.sync.dma_start(
                                o_d[row0:row0 + P, 0:NT], o_sb[:, 0:NT])
                            nc.sync.dma_start(rs_d[qq], rs_cols[:])
                        else:
                            nc.sync.dma_start(
                                o_d[row0:row0 + P, NT:D], o_sb[:, NT:D])
                if not last_blk:
                    nc.vector.tensor_copy(rs_cols[:, qc:qc + 1], psq[:])
                    nc.sync.dma_start(o_d[row0:row0 + P, :], o_sb[:])
            if not last_qq:
                nc.sync.dma_start(rs_d[qq], rs_cols[:])

    nc.compile()
    return nc


_CACHE: dict = {}


def _get_program() -> bass.Bass:
    if "nc" not in _CACHE:
        _CACHE["nc"] = build_program()
    return _CACHE["nc"]


def kernel(x, Wq, bq, Wk, bk, Wv, bv, _trace=False, _trace_kwargs=None):
    nc = _get_program()
    x = np.asarray(x, dtype=np.float32)

    def wrearr(w):
        # [d, e] -> [ec, p(d%128), dc, e%128] so each ec-block DMA is
        # contiguous with 2KB per partition line
        w = np.asarray(w, np.float32).astype(NPBF16)
        return np.ascontiguousarray(
            w.reshape(DC, P, EC, P).transpose(2, 1, 0, 3))

    def brearr(b):
        b = np.asarray(b, np.float32)
        return np.ascontiguousarray(b.reshape(EC, P).T)

    shared = {
        "Wqr": wrearr(Wq),
        "Wkr": wrearr(Wk),
        "Wvr": np.ascontiguousarray(np.asarray(Wv, np.float32).astype(NPBF16)),
        "bqr": brearr(bq),
        "bkr": brearr(bk),
        "bvr": np.ascontiguousarray(
            np.asarray(bv, np.float32).astype(NPBF16).reshape(1, D)),
    }
    in_maps = []
    for c in range(8):
        b, h = divmod(c, 2)
        xb = x[b]
        if h:
            xb = np.roll(xb, -SK, axis=0)  # own key half first
        xTb = xb.T.astype(NPBF16)  # [D, S]
        # [d, s] -> [tb, p, dc, t]: each 512-token block fully contiguous
        xTr = np.ascontiguousarray(
            xTb.reshape(DC, P, S // NT, NT).transpose(2, 1, 0, 3))
        in_maps.append({"xT": xTr, **shared})

    res = run_bass_kernel_spmd(
        nc, in_maps, list(range(8)),
        trace=_trace, **(_trace_kwargs or {}),
    )
    out = np.empty((4, S, D), dtype=np.float32)
    for b in range(4):
        def unrs(r):
            # device writes rs as [qq, p, qc] = rowsum(qq*512 + qc*128 + p)
            return r.reshape(S // QT, P, QT // P).transpose(
                0, 2, 1).reshape(S).astype(np.float64)

        o0 = res.results[2 * b]["o_raw"].astype(np.float64)
        r0 = unrs(res.results[2 * b]["rs_raw"])
        o1 = res.results[2 * b + 1]["o_raw"].astype(np.float64)
        r1 = unrs(res.results[2 * b + 1]["rs_raw"])
        # core h=1 computed queries in rolled order; un-roll before combining
        o1 = np.roll(o1, SK, axis=0)
        r1 = np.roll(r1, SK)
        out[b] = ((o0 + o1) / (r0 + r1)[:, None]).astype(np.float32)
    if _trace:
        return out, res
    return out

